# revision 7
# baseline (speedup 1.0000x reference)
"""Causal self-attention Trainium2 Bass kernel (V5).

Full-input contract: kernel(**inputs) takes the unsharded inputs
(x [8,1024,768], W_attn [768,2304], b_attn [2304], W_proj [768,768],
b_proj [768]) and returns the full output [8,1024,768].

Sharding: data parallel - batch element b runs on NeuronCore b (B=8 =
n_cores), no collectives needed.

V5 changes vs V4 (trace-driven; V4 331us, PE 60% cov, HAM throttled to
K=4/8 for the entire 188us attention phase, 40us DVE reciprocal, 88us
ACT exp):
  - host-side prep: x is transposed and cast to bf16 on the host
    (xT input [768,1024]); W_attn split into Wq/Wk/Wv and cast bf16;
    W_proj bf16. Kills the 48 PE transposes + DVE copies of phase 1 and
    halves weight DMA bytes.
  - all GEMMs run on bf16 operands (fp32 PSUM accumulation).
  - attention pipeline unit is a (k-tile, col-half) CHUNK with fp32
    scores in a [128, 2x512] PSUM tile (2 banks, both heads of the
    pair). Chunks are double-buffered (4 banks) next to the 4 avp
    banks, so the score MM for chunk n+2 no longer waits on exp(n):
    the PE never idles long enough for HAM to re-throttle.
  - one exp per chunk covers both heads ([128, 2, n] AP) - halves ACT
    instruction count; one affine_select masks both heads' diagonal.
  - softmax divide: l-rows gathered by SBUF->SBUF DMA into a [4,512]
    tile per head pair, ONE reciprocal_approx_fast (the V4 kernel spent
    40us in 12 full-precision Newton reciprocals), DMA broadcast,
    DVE multiply fused into yT (bf16).
"""

import os
import sys

import numpy as np

for _p in ("/opt/trn_rl_repo", "/root/.axon_site/_ro/trn_rl_repo"):
    if os.path.isdir(_p) and _p not in sys.path:
        sys.path.insert(0, _p)
        break

import concourse.bass as bass
import concourse.mybir as mybir
import concourse.tile as tile
from concourse.bass_utils import run_bass_kernel_spmd

T, C, H = 1024, 768, 12
C3 = 3 * C
NCORES = 8
NT = T // 128    # 8 t-tiles
NC_ = C // 128   # 6 c-tiles
NHP = H // 2     # 6 head pairs
f32 = mybir.dt.float32
bf16 = mybir.dt.bfloat16

EXP = mybir.ActivationFunctionType.Exp


def build_module():
    nc = bass.Bass()
    xT_d = nc.dram_tensor("xT", [C, T], bf16, kind="ExternalInput")
    wq_d = nc.dram_tensor("Wq", [C, C], bf16, kind="ExternalInput")
    wk_d = nc.dram_tensor("Wk", [C, C], bf16, kind="ExternalInput")
    wv_d = nc.dram_tensor("Wv", [C, C], bf16, kind="ExternalInput")
    wp_d = nc.dram_tensor("Wp", [C, C], bf16, kind="ExternalInput")
    ba_d = nc.dram_tensor("b_attn", [1, C3], f32, kind="ExternalInput")
    bp_d = nc.dram_tensor("b_proj", [1, C], f32, kind="ExternalInput")
    out_d = nc.dram_tensor("out", [T, C], f32, kind="ExternalOutput")

    with tile.TileContext(nc) as tc:
        with tc.tile_pool(name="persist", bufs=1) as P0:
            qkT = [P0.tile([128, T], bf16, name=f"qkT{m}") for m in range(2 * NC_)]
            vA = [P0.tile([128, 65 * H], bf16, name=f"vA{t}") for t in range(NT)]
            yT = [P0.tile([128, T], bf16, name=f"yT{c}") for c in range(NC_)]
            ba_sb = P0.tile([1, C], f32, name="ba_sb")
            bp_sb = P0.tile([1, C], f32, name="bp_sb")
            baB = P0.tile([128, C], f32, name="baB")   # b_attn v-part bcast
            bpB = P0.tile([128, C], f32, name="bpB")   # b_proj bcast
            wpt = [P0.tile([128, C], bf16, name=f"wp{c}") for c in range(NC_)]
            bqk = [P0.tile([128, 1], f32, name=f"bqk{m}") for m in range(2 * NC_)]
            ones_col = P0.tile([128, H], bf16, name="ones_col")
            nc.vector.memset(ones_col[:], 1.0)
            warm_src = P0.tile([1, 16], f32, name="warm_src")
            nc.vector.memset(warm_src[:], 1.0)

            # preload the exp table while ACT is idle (else the first
            # attention exp pays the ~2.7us ACT_TABLE_LOAD inline)
            warm = P0.tile([1, 16], f32, name="warm")
            nc.scalar.activation(warm[:], warm_src[:], EXP, scale=0.125)

            # ---- phase A: qkv GEMMs (x arrives pre-transposed bf16) ----
            with tc.tile_pool(name="sbA", bufs=1) as SBA:
                xT = [SBA.tile([128, T], bf16, name=f"xT{c}", tag=f"xT{c}",
                               bufs=1) for c in range(NC_)]
                wV = [SBA.tile([128, C], bf16, name=f"wV{c}", tag=f"wV{c}",
                               bufs=1) for c in range(NC_)]
                # interleave x/weight loads across both HWDGE queues so the
                # first v-GEMM accumulation chain can start ~2 tiles in
                nc.sync.dma_start(out=ba_sb[:], in_=ba_d[0:1, 2 * C:3 * C])
                for c in range(NC_):
                    q = nc.sync if c % 2 == 0 else nc.scalar
                    q.dma_start(out=xT[c][:],
                                in_=xT_d[128 * c:128 * (c + 1), :])
                    q.dma_start(out=wV[c][:],
                                in_=wv_d[128 * c:128 * (c + 1), :])
                # one-time bias broadcast (free-dim stride-0 DMA replicate)
                # on the gpsimd SWDGE queue (descriptor gen on Q7, off both
                # HWDGE queues)
                nc.gpsimd.dma_start(
                    out=baB[:],
                    in_=ba_sb[0:1, :].unsqueeze(1).to_broadcast([1, 128, C]))
                baB_r = baB.rearrange("p (h e) -> p h e", h=H)
                # bqk partition-scatter DMAs (4B-granular, slow to issue) on
                # the gpsimd SWDGE queue, off the weight-load path
                for m in range(2 * NC_):
                    nc.gpsimd.dma_start(
                        out=bqk[m][:],
                        in_=ba_d[0:1, 128 * m:128 * (m + 1)]
                            .rearrange("a p -> p a"))
                # q/k weight loads stream behind the v weights
                wQ = [SBA.tile([128, C], bf16, name=f"wQ{c}", tag=f"wQ{c}",
                               bufs=1) for c in range(NC_)]
                wK = [SBA.tile([128, C], bf16, name=f"wK{c}", tag=f"wK{c}",
                               bufs=1) for c in range(NC_)]
                for c in range(NC_):
                    q = nc.sync if c % 2 == 0 else nc.scalar
                    q.dma_start(out=wQ[c][:],
                                in_=wq_d[128 * c:128 * (c + 1), :])
                for c in range(NC_):
                    q = nc.sync if c % 2 == 0 else nc.scalar
                    q.dma_start(out=wK[c][:],
                                in_=wk_d[128 * c:128 * (c + 1), :])

                with tc.tile_pool(name="psA", bufs=1, space="PSUM") as PSA:
                    # v: stationary xT columns, moving W_v rows
                    for t in range(NT):
                        accv = PSA.tile([128, C], f32, tag="v", bufs=2,
                                        name="accv")
                        for c in range(NC_):
                            xcol = xT[c][:, 128 * t:128 * (t + 1)]
                            nc.tensor.matmul(accv[:, 0:512], xcol,
                                             wV[c][:, 0:512],
                                             start=(c == 0), stop=(c == NC_ - 1))
                            nc.tensor.matmul(accv[:, 512:C], xcol,
                                             wV[c][:, 512:C],
                                             start=(c == 0), stop=(c == NC_ - 1))
                        av = vA[t].rearrange("p (h e) -> p h e", h=H)
                        nc.vector.tensor_copy(
                            av[:, :, 64:65],
                            ones_col.rearrange("p (h o) -> p h o", o=1))
                        # eviction with fused bias add
                        nc.vector.tensor_tensor(
                            av[:, :, 0:64],
                            accv[:].rearrange("p (h e) -> p h e", h=H),
                            baB_r[:, :, 0:64],
                            mybir.AluOpType.add)

                    # q^T / k^T: out rows = W columns (m-tile), moving xT
                    for half in range(2):
                        wh = wQ if half == 0 else wK
                        for mm in range(NC_):
                            m = NC_ * half + mm
                            acc = PSA.tile([128, T], f32, tag="qk", bufs=2,
                                           name="acc")
                            for c in range(NC_):
                                wa = wh[c][:, 128 * mm:128 * (mm + 1)]
                                for j2 in range(2):
                                    nc.tensor.matmul(
                                        acc[:, 512 * j2:512 * (j2 + 1)],
                                        wa,
                                        xT[c][:, 512 * j2:512 * (j2 + 1)],
                                        start=(c == 0), stop=(c == NC_ - 1),
                                    )
                            # psum -> sbuf(bf16) with per-partition bias add
                            nc.vector.tensor_scalar_add(qkT[m][:], acc[:],
                                                        bqk[m][:])

            # ---- phase B: attention ----
            # W_proj / b_proj loads issued here: sync queue is idle now and
            # phase C needs them much later
            nc.sync.dma_start(out=bp_sb[:], in_=bp_d[:])
            for c in range(NC_):
                q = nc.sync if c % 2 == 0 else nc.scalar
                q.dma_start(out=wpt[c][:],
                            in_=wp_d[128 * c:128 * (c + 1), :])
            nc.gpsimd.dma_start(
                out=bpB[:],
                in_=bp_sb[0:1, :].unsqueeze(1).to_broadcast([1, 128, C]))

            with tc.tile_pool(name="ps3", bufs=1, space="PSUM") as PS3, \
                 tc.tile_pool(name="sb3", bufs=1) as SB3:
                from collections import deque
                pending = deque()   # deferred normalization pipeline stages

                def pop_pending(k=2):
                    n = 0
                    while pending and n < k:
                        s = pending.popleft()
                        if s is not None:
                            s()
                        n += 1

                # chunk list: (i, w); w=0 -> query cols [lo,512) (i<4 only),
                # w=1 -> [max(lo,512), 1024)
                chunks = []
                for i in range(NT):
                    if i < 4:
                        chunks.append((i, 0))
                        chunks.append((i, 1))
                    else:
                        chunks.append((i, 1))

                def chunk_cols(i, w):
                    lo = 128 * i
                    if w == 0:
                        return lo, 512
                    return max(lo, 512), T

                for hp in range(NHP):
                    qt = qkT[hp]
                    kt = qkT[NC_ + hp]
                    sps = {}   # chunk -> score PSUM tile
                    pbs = {}   # chunk -> exp'd SBUF tile
                    avs = {}   # (hs, half) -> [65,512] accumulator
                    st = {}    # (hs, half) -> staged yU tile

                    def emit_score(ch):
                        i, w = ch
                        lo = 128 * i
                        c0, c1 = chunk_cols(i, w)
                        scp = PS3.tile([128, 1024], f32, tag="sc", bufs=2,
                                       name="scp")
                        for hs in range(2):
                            base = 64 * hs
                            nc.tensor.matmul(
                                scp[:, 512 * hs:512 * hs + (c1 - c0)],
                                kt[base:base + 64, lo:lo + 128],
                                qt[base:base + 64, c0:c1],
                                start=True, stop=True)
                        sps[ch] = scp

                    def emit_exp(ch):
                        i, w = ch
                        lo = 128 * i
                        c0, c1 = chunk_cols(i, w)
                        n = c1 - c0
                        scp = sps.pop(ch)
                        pb = SB3.tile([128, 1024], bf16, tag="pb", bufs=4,
                                      name="pb")
                        scv = scp.rearrange("p (s n) -> p s n", s=2)
                        pbv = pb.rearrange("p (s n) -> p s n", s=2)
                        nc.scalar.activation(pbv[:, :, 0:n], scv[:, :, 0:n],
                                             EXP, scale=0.125)
                        if c0 == lo:
                            # diagonal [128,128] block (both heads):
                            # keep iff q - key >= 0
                            nc.gpsimd.affine_select(
                                out=pbv[:, :, 0:128], in_=pbv[:, :, 0:128],
                                pattern=[[0, 2], [1, 128]],
                                compare_op=mybir.AluOpType.is_ge, fill=0.0,
                                base=0, channel_multiplier=-1,
                            )
                        pbs[ch] = pb

                    def emit_av(ch):
                        i, w = ch
                        c0, c1 = chunk_cols(i, w)
                        n = c1 - c0
                        pb = pbs.pop(ch)
                        if i == 0 and w == 0:
                            for hs in range(2):
                                for half in range(2):
                                    avs[(hs, half)] = PS3.tile(
                                        [65, 512], f32, tag=f"av{hs}{half}",
                                        bufs=1, name=f"av{hs}{half}")
                        for hs in range(2):
                            h = 2 * hp + hs
                            vt = vA[i][:, 65 * h:65 * h + 65]
                            if w == 0:
                                nc.tensor.matmul(
                                    avs[(hs, 0)][0:65, c0:512], vt,
                                    pb[:, 512 * hs:512 * hs + n],
                                    start=(i == 0), stop=(i == 3),
                                    skip_group_check=True)
                            else:
                                nc.tensor.matmul(
                                    avs[(hs, 1)][0:65, c0 - 512:512], vt,
                                    pb[:, 512 * hs:512 * hs + n],
                                    start=(i == 0), stop=(i == NT - 1),
                                    skip_group_check=True)

                    def make_s0(myst, myavs, half):
                        def s0():   # PSUM -> SBUF staging (frees the banks)
                            for hs in range(2):
                                yU = SB3.tile([65, 512], f32,
                                              tag=f"yU{hs}{half}", bufs=2,
                                              name=f"yU{hs}{half}")
                                nc.vector.tensor_copy(
                                    yU[:], myavs[(hs, half)][0:65, 0:512])
                                myst[(hs, half)] = yU
                        return s0

                    def make_norm(myhp, myst):
                        # l-row gather (DMA), one fast reciprocal, DMA
                        # broadcast, DVE multiply into yT
                        loc = {}

                        def s_gather():
                            # l-rows packed [hs, 512*half + q]
                            lr = SB3.tile([2, T], f32, tag="lr", bufs=2,
                                          name="lr")
                            for hs in range(2):
                                for half in range(2):
                                    nc.sync.dma_start(
                                        out=lr[hs:hs + 1,
                                               512 * half:512 * (half + 1)],
                                        in_=myst[(hs, half)][64:65, :])
                            loc["lr"] = lr

                        def s_recip():
                            rl = SB3.tile([2, T], f32, tag="rl", bufs=2,
                                          name="rl")
                            nc.vector.reciprocal_approx_fast(rl[:],
                                                             loc["lr"][:])
                            loc["rl"] = rl

                        def s_bcast():
                            # partition replicate on the gpsimd SWDGE queue
                            # (HWDGE-issued replicates cost up to 5us of
                            # sequencer time and stalled the exp pipeline)
                            for hs in range(2):
                                rlb = SB3.tile([64, T], f32,
                                               tag=f"rlb{hs}", bufs=2,
                                               name=f"rlb{hs}")
                                nc.gpsimd.dma_start(
                                    out=rlb[:],
                                    in_=loc["rl"][hs:hs + 1, :].unsqueeze(1)
                                        .to_broadcast([1, 64, T]))
                                loc[("rlb", hs)] = rlb

                        def s_mult():
                            for hs in range(2):
                                base = 64 * hs
                                for half in range(2):
                                    nc.vector.tensor_tensor(
                                        yT[myhp][base:base + 64,
                                                 512 * half:512 * (half + 1)],
                                        myst[(hs, half)][0:64, :],
                                        loc[("rlb", hs)][:,
                                            512 * half:512 * (half + 1)],
                                        mybir.AluOpType.mult)

                        return [s_gather, None, s_recip, s_bcast, None,
                                s_mult]

                    # software-pipelined emission: scores one chunk ahead of
                    # the AV stream; deferred norm stages drain in the gaps
                    pop_pending()
                    emit_score(chunks[0])
                    emit_exp(chunks[0])
                    for n_ in range(1, len(chunks)):
                        pop_pending()
                        emit_score(chunks[n_])
                        emit_exp(chunks[n_])
                        emit_av(chunks[n_ - 1])
                        if chunks[n_ - 1] == (3, 0):
                            pending.append(make_s0(st, avs, 0))
                    emit_av(chunks[-1])
                    pending.append(make_s0(st, avs, 1))
                    pending.extend(make_norm(hp, st))

                # drain the tail (last pair's normalization)
                while pending:
                    s = pending.popleft()
                    if s is not None:
                        s()

            # ---- phase C: out = y^T.T @ W_proj + b_proj ----
            with tc.tile_pool(name="ps4", bufs=2, space="PSUM") as PS4, \
                 tc.tile_pool(name="sb4", bufs=3) as SB4:
                for t in range(NT):
                    acc = PS4.tile([128, C], f32, tag="pj", name="acc")
                    for c in range(NC_):
                        ycol = yT[c][:, 128 * t:128 * (t + 1)]
                        nc.tensor.matmul(acc[:, 0:512], ycol,
                                         wpt[c][:, 0:512],
                                         start=(c == 0), stop=(c == NC_ - 1))
                        nc.tensor.matmul(acc[:, 512:C], ycol,
                                         wpt[c][:, 512:C],
                                         start=(c == 0), stop=(c == NC_ - 1))
                    ot = SB4.tile([128, C], f32, tag="ot", bufs=3, name="ot")
                    nc.vector.tensor_tensor(ot[:], acc[:], bpB[:],
                                            mybir.AluOpType.add)
                    nc.sync.dma_start(out=out_d[128 * t:128 * (t + 1), :],
                                      in_=ot[:])

    return nc


_WAIT_SKIP = {"InstNoOp", "InstEventSemOp", "InstSemaphoreOp",
              "InstPartitionBroadcast", "InstPartitionAllReduce"}


def _legalize_waits(nc):
    """walrus's codegen allows limited sync-wait commands per ISA struct
    (e.g. a Matmult's waits all land on the generated LDWEIGHTS struct which
    has one slot). Move excess waits onto same-engine NoOps inserted
    immediately before the instruction - program order on the engine queue
    preserves the synchronization semantics."""
    nfix = 0
    for fn in nc.m.functions:
        for bb in fn.blocks:
            out = []
            for ins in bb.instructions:
                si = ins.sync_info
                if (type(ins).__name__ not in _WAIT_SKIP and si is not None
                        and si.on_wait and len(si.on_wait) > 1):
                    waits = list(si.on_wait)
                    extra, keep = waits[:-1], waits[-1:]
                    for k, w in enumerate(extra):
                        nop = mybir.InstNoOp(name=f"{ins.name}-wf{k}", ins=[],
                                             outs=[])
                        nop.engine = ins.engine
                        nop.sync_info = mybir.SyncInfo(on_wait=[w],
                                                       on_update=[])
                        out.append(nop)
                    ins.sync_info = mybir.SyncInfo(
                        on_wait=keep, on_update=list(si.on_update or []))
                    nfix += 1
                out.append(ins)
            bb.instructions = out
    return nfix


_cached_module = None


def _get_module():
    global _cached_module
    if _cached_module is None:
        nc = build_module()
        # populate .instr bytes for InstCustomDveAnt (reciprocal_approx_fast)
        # - Bacc.compile() runs this pass but the raw-Bass path doesn't, and
        # walrus codegen fails with "ISA wrong length" on empty .instr
        mybir.codegen_inst_isa_subclasses(nc)
        _legalize_waits(nc)
        _cached_module = nc
    return _cached_module


def make_in_maps(x, W_attn, b_attn, W_proj, b_proj):
    import ml_dtypes
    bf = ml_dtypes.bfloat16
    x = np.asarray(x, dtype=np.float32)
    wa = np.asarray(W_attn, dtype=np.float32)
    wq = np.ascontiguousarray(wa[:, 0:C].astype(bf))
    wk = np.ascontiguousarray(wa[:, C:2 * C].astype(bf))
    wv = np.ascontiguousarray(wa[:, 2 * C:3 * C].astype(bf))
    wp = np.ascontiguousarray(np.asarray(W_proj, dtype=np.float32).astype(bf))
    ba = np.ascontiguousarray(
        np.asarray(b_attn, dtype=np.float32).reshape(1, C3))
    bp = np.ascontiguousarray(
        np.asarray(b_proj, dtype=np.float32).reshape(1, C))
    return [
        dict(xT=np.ascontiguousarray(x[b].T.astype(bf)),
             Wq=wq, Wk=wk, Wv=wv, Wp=wp, b_attn=ba, b_proj=bp)
        for b in range(x.shape[0])
    ]


def run(x, W_attn, b_attn, W_proj, b_proj, trace=False, **spmd_kwargs):
    nc = _get_module()
    in_maps = make_in_maps(x, W_attn, b_attn, W_proj, b_proj)
    res = run_bass_kernel_spmd(nc, in_maps, list(range(NCORES)), trace=trace,
                               **spmd_kwargs)
    out = np.stack([res.results[b]["out"] for b in range(len(in_maps))],
                   axis=0)
    return out, res


def kernel(x, W_attn, b_attn, W_proj, b_proj):
    out, _ = run(x, W_attn, b_attn, W_proj, b_proj)
    return out


# revision 9
# speedup vs baseline: 1.1686x; 1.1686x over previous
"""Causal self-attention Trainium2 Bass kernel (V5).

Full-input contract: kernel(**inputs) takes the unsharded inputs
(x [8,1024,768], W_attn [768,2304], b_attn [2304], W_proj [768,768],
b_proj [768]) and returns the full output [8,1024,768].

Sharding: data parallel - batch element b runs on NeuronCore b (B=8 =
n_cores), no collectives needed.

V5 changes vs V4 (trace-driven; V4 331us, PE 60% cov, HAM throttled to
K=4/8 for the entire 188us attention phase, 40us DVE reciprocal, 88us
ACT exp):
  - host-side prep: x is transposed and cast to bf16 on the host
    (xT input [768,1024]); W_attn split into Wq/Wk/Wv and cast bf16;
    W_proj bf16. Kills the 48 PE transposes + DVE copies of phase 1 and
    halves weight DMA bytes.
  - all GEMMs run on bf16 operands (fp32 PSUM accumulation).
  - attention pipeline unit is a (k-tile, col-half) CHUNK with fp32
    scores in a [128, 2x512] PSUM tile (2 banks, both heads of the
    pair). Chunks are double-buffered (4 banks) next to the 4 avp
    banks, so the score MM for chunk n+2 no longer waits on exp(n):
    the PE never idles long enough for HAM to re-throttle.
  - one exp per chunk covers both heads ([128, 2, n] AP) - halves ACT
    instruction count; one affine_select masks both heads' diagonal.
  - softmax divide: l-rows gathered by SBUF->SBUF DMA into a [4,512]
    tile per head pair, ONE reciprocal_approx_fast (the V4 kernel spent
    40us in 12 full-precision Newton reciprocals), DMA broadcast,
    DVE multiply fused into yT (bf16).
"""

import os
import sys

import numpy as np

for _p in ("/opt/trn_rl_repo", "/root/.axon_site/_ro/trn_rl_repo"):
    if os.path.isdir(_p) and _p not in sys.path:
        sys.path.insert(0, _p)
        break

import concourse.bass as bass
import concourse.mybir as mybir
import concourse.tile as tile
from concourse.bass_utils import run_bass_kernel_spmd

T, C, H = 1024, 768, 12
C3 = 3 * C
NCORES = 8
NT = T // 128    # 8 t-tiles
NC_ = C // 128   # 6 c-tiles
NHP = H // 2     # 6 head pairs
f32 = mybir.dt.float32
bf16 = mybir.dt.bfloat16

EXP = mybir.ActivationFunctionType.Exp


def build_module():
    nc = bass.Bass()
    xT_d = nc.dram_tensor("xT", [C, T], bf16, kind="ExternalInput")
    wq_d = nc.dram_tensor("Wq", [C, C], bf16, kind="ExternalInput")
    wk_d = nc.dram_tensor("Wk", [C, C], bf16, kind="ExternalInput")
    wv_d = nc.dram_tensor("Wv", [C, C], bf16, kind="ExternalInput")
    wp_d = nc.dram_tensor("Wp", [C, C], bf16, kind="ExternalInput")
    ba_d = nc.dram_tensor("b_attn", [1, C3], f32, kind="ExternalInput")
    bp_d = nc.dram_tensor("b_proj", [1, C], f32, kind="ExternalInput")
    out_d = nc.dram_tensor("out", [T, C], f32, kind="ExternalOutput")

    with tile.TileContext(nc) as tc:
        with tc.tile_pool(name="persist", bufs=1) as P0:
            qkT = [P0.tile([128, T], bf16, name=f"qkT{m}") for m in range(2 * NC_)]
            vA = [P0.tile([128, 65 * H], bf16, name=f"vA{t}") for t in range(NT)]
            yT = [P0.tile([128, T], bf16, name=f"yT{c}") for c in range(NC_)]
            ba_sb = P0.tile([1, C], f32, name="ba_sb")
            bp_sb = P0.tile([1, C], f32, name="bp_sb")
            baB = P0.tile([128, C], f32, name="baB")   # b_attn v-part bcast
            bpB = P0.tile([128, C], f32, name="bpB")   # b_proj bcast
            wpt = [P0.tile([128, C], bf16, name=f"wp{c}") for c in range(NC_)]
            bqk = [P0.tile([128, 1], f32, name=f"bqk{m}") for m in range(2 * NC_)]
            ones_col = P0.tile([128, H], bf16, name="ones_col")
            nc.vector.memset(ones_col[:], 1.0)
            warm_src = P0.tile([1, 16], f32, name="warm_src")
            nc.vector.memset(warm_src[:], 1.0)

            # preload the exp table while ACT is idle (else the first
            # attention exp pays the ~2.7us ACT_TABLE_LOAD inline)
            warm = P0.tile([1, 16], f32, name="warm")
            nc.scalar.activation(warm[:], warm_src[:], EXP, scale=0.125)

            # ---- phase A: qkv GEMMs (x arrives pre-transposed bf16) ----
            with tc.tile_pool(name="sbA", bufs=1) as SBA:
                xT = [SBA.tile([128, T], bf16, name=f"xT{c}", tag=f"xT{c}",
                               bufs=1) for c in range(NC_)]
                wV = [SBA.tile([128, C], bf16, name=f"wV{c}", tag=f"wV{c}",
                               bufs=1) for c in range(NC_)]
                # interleave x/weight loads across both HWDGE queues so the
                # first v-GEMM accumulation chain can start ~2 tiles in
                nc.sync.dma_start(out=ba_sb[:], in_=ba_d[0:1, 2 * C:3 * C])
                for c in range(NC_):
                    q = nc.sync if c % 2 == 0 else nc.scalar
                    q.dma_start(out=xT[c][:],
                                in_=xT_d[128 * c:128 * (c + 1), :])
                    q.dma_start(out=wV[c][:],
                                in_=wv_d[128 * c:128 * (c + 1), :])
                # one-time bias broadcast (free-dim stride-0 DMA replicate)
                # on the gpsimd SWDGE queue (descriptor gen on Q7, off both
                # HWDGE queues)
                nc.gpsimd.dma_start(
                    out=baB[:],
                    in_=ba_sb[0:1, :].unsqueeze(1).to_broadcast([1, 128, C]))
                baB_r = baB.rearrange("p (h e) -> p h e", h=H)
                # bqk partition-scatter DMAs (4B-granular, slow to issue) on
                # the gpsimd SWDGE queue, off the weight-load path
                for m in range(2 * NC_):
                    nc.gpsimd.dma_start(
                        out=bqk[m][:],
                        in_=ba_d[0:1, 128 * m:128 * (m + 1)]
                            .rearrange("a p -> p a"))
                # q/k weight loads stream behind the v weights
                wQ = [SBA.tile([128, C], bf16, name=f"wQ{c}", tag=f"wQ{c}",
                               bufs=1) for c in range(NC_)]
                wK = [SBA.tile([128, C], bf16, name=f"wK{c}", tag=f"wK{c}",
                               bufs=1) for c in range(NC_)]
                for c in range(NC_):
                    q = nc.sync if c % 2 == 0 else nc.scalar
                    q.dma_start(out=wQ[c][:],
                                in_=wq_d[128 * c:128 * (c + 1), :])
                for c in range(NC_):
                    q = nc.sync if c % 2 == 0 else nc.scalar
                    q.dma_start(out=wK[c][:],
                                in_=wk_d[128 * c:128 * (c + 1), :])

                with tc.tile_pool(name="psA", bufs=1, space="PSUM") as PSA:
                    # v: stationary xT columns, moving W_v rows
                    for t in range(NT):
                        accv = PSA.tile([128, C], f32, tag="v", bufs=2,
                                        name="accv")
                        for c in range(NC_):
                            xcol = xT[c][:, 128 * t:128 * (t + 1)]
                            nc.tensor.matmul(accv[:, 0:512], xcol,
                                             wV[c][:, 0:512],
                                             start=(c == 0), stop=(c == NC_ - 1))
                            nc.tensor.matmul(accv[:, 512:C], xcol,
                                             wV[c][:, 512:C],
                                             start=(c == 0), stop=(c == NC_ - 1))
                        av = vA[t].rearrange("p (h e) -> p h e", h=H)
                        nc.vector.tensor_copy(
                            av[:, :, 64:65],
                            ones_col.rearrange("p (h o) -> p h o", o=1))
                        # eviction with fused bias add
                        nc.vector.tensor_tensor(
                            av[:, :, 0:64],
                            accv[:].rearrange("p (h e) -> p h e", h=H),
                            baB_r[:, :, 0:64],
                            mybir.AluOpType.add)

                    # q^T / k^T: out rows = W columns (m-tile), moving xT
                    for half in range(2):
                        wh = wQ if half == 0 else wK
                        for mm in range(NC_):
                            m = NC_ * half + mm
                            acc = PSA.tile([128, T], f32, tag="qk", bufs=2,
                                           name="acc")
                            for c in range(NC_):
                                wa = wh[c][:, 128 * mm:128 * (mm + 1)]
                                for j2 in range(2):
                                    nc.tensor.matmul(
                                        acc[:, 512 * j2:512 * (j2 + 1)],
                                        wa,
                                        xT[c][:, 512 * j2:512 * (j2 + 1)],
                                        start=(c == 0), stop=(c == NC_ - 1),
                                    )
                            # psum -> sbuf(bf16) with per-partition bias add
                            nc.vector.tensor_scalar_add(qkT[m][:], acc[:],
                                                        bqk[m][:])

            # ---- phase B: attention ----
            # W_proj / b_proj loads issued here: sync queue is idle now and
            # phase C needs them much later
            nc.sync.dma_start(out=bp_sb[:], in_=bp_d[:])
            for c in range(NC_):
                q = nc.sync if c % 2 == 0 else nc.scalar
                q.dma_start(out=wpt[c][:],
                            in_=wp_d[128 * c:128 * (c + 1), :])
            nc.gpsimd.dma_start(
                out=bpB[:],
                in_=bp_sb[0:1, :].unsqueeze(1).to_broadcast([1, 128, C]))

            with tc.tile_pool(name="ps3", bufs=1, space="PSUM") as PS3, \
                 tc.tile_pool(name="sb3", bufs=1) as SB3:
                from collections import deque
                pending = deque()   # deferred normalization pipeline stages

                def pop_pending(k=2):
                    n = 0
                    while pending and n < k:
                        s = pending.popleft()
                        if s is not None:
                            s()
                        n += 1

                # chunk list: (i, w); w=0 -> query cols [lo,512) (i<4 only),
                # w=1 -> [max(lo,512), 1024)
                chunks = []
                for i in range(NT):
                    if i < 4:
                        chunks.append((i, 0))
                        chunks.append((i, 1))
                    else:
                        chunks.append((i, 1))

                def chunk_cols(i, w):
                    lo = 128 * i
                    if w == 0:
                        return lo, 512
                    return max(lo, 512), T

                for hp in range(NHP):
                    qt = qkT[hp]
                    kt = qkT[NC_ + hp]
                    sps = {}   # chunk -> score PSUM tile
                    pbs = {}   # chunk -> exp'd SBUF tile
                    avs = {}   # (hs, half) -> [65,512] accumulator
                    st = {}    # (hs, half) -> staged yU tile

                    def emit_score(ch):
                        i, w = ch
                        lo = 128 * i
                        c0, c1 = chunk_cols(i, w)
                        scp = PS3.tile([128, 1024], f32, tag="sc", bufs=2,
                                       name="scp")
                        for hs in range(2):
                            base = 64 * hs
                            nc.tensor.matmul(
                                scp[:, 512 * hs:512 * hs + (c1 - c0)],
                                kt[base:base + 64, lo:lo + 128],
                                qt[base:base + 64, c0:c1],
                                start=True, stop=True)
                        sps[ch] = scp

                    def emit_exp(ch):
                        i, w = ch
                        lo = 128 * i
                        c0, c1 = chunk_cols(i, w)
                        n = c1 - c0
                        scp = sps.pop(ch)
                        pb = SB3.tile([128, 1024], bf16, tag="pb", bufs=4,
                                      name="pb")
                        scv = scp.rearrange("p (s n) -> p s n", s=2)
                        pbv = pb.rearrange("p (s n) -> p s n", s=2)
                        nc.scalar.activation(pbv[:, :, 0:n], scv[:, :, 0:n],
                                             EXP, scale=0.125)
                        if c0 == lo:
                            # diagonal [128,128] block (both heads):
                            # keep iff q - key >= 0
                            nc.gpsimd.affine_select(
                                out=pbv[:, :, 0:128], in_=pbv[:, :, 0:128],
                                pattern=[[0, 2], [1, 128]],
                                compare_op=mybir.AluOpType.is_ge, fill=0.0,
                                base=0, channel_multiplier=-1,
                            )
                        pbs[ch] = pb

                    def emit_av(ch):
                        i, w = ch
                        c0, c1 = chunk_cols(i, w)
                        n = c1 - c0
                        pb = pbs.pop(ch)
                        if i == 0 and w == 0:
                            for hs in range(2):
                                for half in range(2):
                                    avs[(hs, half)] = PS3.tile(
                                        [65, 512], f32, tag=f"av{hs}{half}",
                                        bufs=1, name=f"av{hs}{half}")
                        for hs in range(2):
                            h = 2 * hp + hs
                            vt = vA[i][:, 65 * h:65 * h + 65]
                            if w == 0:
                                nc.tensor.matmul(
                                    avs[(hs, 0)][0:65, c0:512], vt,
                                    pb[:, 512 * hs:512 * hs + n],
                                    start=(i == 0), stop=(i == 3),
                                    skip_group_check=True)
                            else:
                                nc.tensor.matmul(
                                    avs[(hs, 1)][0:65, c0 - 512:512], vt,
                                    pb[:, 512 * hs:512 * hs + n],
                                    start=(i == 0), stop=(i == NT - 1),
                                    skip_group_check=True)

                    def make_s0(myst, myavs, half):
                        def s0():   # PSUM -> SBUF staging (frees the banks)
                            for hs in range(2):
                                yU = SB3.tile([65, 512], f32,
                                              tag=f"yU{hs}{half}", bufs=2,
                                              name=f"yU{hs}{half}")
                                nc.vector.tensor_copy(
                                    yU[:], myavs[(hs, half)][0:65, 0:512])
                                myst[(hs, half)] = yU
                        return s0

                    def make_norm(myhp, myst):
                        # l-row gather (DMA), one fast reciprocal, DMA
                        # broadcast, DVE multiply into yT
                        loc = {}

                        def s_gather():
                            # l-rows packed [hs, 512*half + q]
                            lr = SB3.tile([2, T], f32, tag="lr", bufs=2,
                                          name="lr")
                            for hs in range(2):
                                for half in range(2):
                                    nc.sync.dma_start(
                                        out=lr[hs:hs + 1,
                                               512 * half:512 * (half + 1)],
                                        in_=myst[(hs, half)][64:65, :])
                            loc["lr"] = lr

                        def s_recip():
                            rl = SB3.tile([2, T], f32, tag="rl", bufs=2,
                                          name="rl")
                            nc.vector.reciprocal_approx_fast(rl[:],
                                                             loc["lr"][:])
                            loc["rl"] = rl

                        def s_bcast():
                            # partition replicate on the sync HWDGE queue
                            # (idle during attention; the scalar queue would
                            # stall exp, the gpsimd queue stalls affine via
                            # post-SWDGE drains)
                            for hs in range(2):
                                rlb = SB3.tile([64, T], f32,
                                               tag=f"rlb{hs}", bufs=2,
                                               name=f"rlb{hs}")
                                nc.sync.dma_start(
                                    out=rlb[:],
                                    in_=loc["rl"][hs:hs + 1, :].unsqueeze(1)
                                        .to_broadcast([1, 64, T]))
                                loc[("rlb", hs)] = rlb

                        def s_mult():
                            for hs in range(2):
                                base = 64 * hs
                                for half in range(2):
                                    nc.vector.tensor_tensor(
                                        yT[myhp][base:base + 64,
                                                 512 * half:512 * (half + 1)],
                                        myst[(hs, half)][0:64, :],
                                        loc[("rlb", hs)][:,
                                            512 * half:512 * (half + 1)],
                                        mybir.AluOpType.mult)

                        return [s_gather, None, s_recip, s_bcast, None,
                                s_mult]

                    # software-pipelined emission: scores TWO chunks ahead of
                    # the AV stream so the in-order PE queue always has a
                    # score MM to run while AV(n) waits on exp/affine(n);
                    # deferred norm stages drain in the gaps
                    nch = len(chunks)
                    pop_pending()
                    emit_score(chunks[0])
                    emit_exp(chunks[0])
                    emit_score(chunks[1])
                    emit_exp(chunks[1])
                    for n_ in range(2, nch):
                        pop_pending()
                        emit_score(chunks[n_])
                        emit_exp(chunks[n_])
                        emit_av(chunks[n_ - 2])
                        if chunks[n_ - 2] == (3, 0):
                            pending.append(make_s0(st, avs, 0))
                    pop_pending()
                    emit_av(chunks[nch - 2])
                    emit_av(chunks[nch - 1])
                    pending.append(make_s0(st, avs, 1))
                    pending.extend(make_norm(hp, st))

                # drain the tail (last pair's normalization)
                while pending:
                    s = pending.popleft()
                    if s is not None:
                        s()

            # ---- phase C: out = y^T.T @ W_proj + b_proj ----
            with tc.tile_pool(name="ps4", bufs=2, space="PSUM") as PS4, \
                 tc.tile_pool(name="sb4", bufs=3) as SB4:
                for t in range(NT):
                    acc = PS4.tile([128, C], f32, tag="pj", name="acc")
                    for c in range(NC_):
                        ycol = yT[c][:, 128 * t:128 * (t + 1)]
                        nc.tensor.matmul(acc[:, 0:512], ycol,
                                         wpt[c][:, 0:512],
                                         start=(c == 0), stop=(c == NC_ - 1))
                        nc.tensor.matmul(acc[:, 512:C], ycol,
                                         wpt[c][:, 512:C],
                                         start=(c == 0), stop=(c == NC_ - 1))
                    ot = SB4.tile([128, C], f32, tag="ot", bufs=3, name="ot")
                    nc.vector.tensor_tensor(ot[:], acc[:], bpB[:],
                                            mybir.AluOpType.add)
                    nc.sync.dma_start(out=out_d[128 * t:128 * (t + 1), :],
                                      in_=ot[:])

    return nc


_WAIT_SKIP = {"InstNoOp", "InstEventSemOp", "InstSemaphoreOp",
              "InstPartitionBroadcast", "InstPartitionAllReduce"}


def _legalize_waits(nc):
    """walrus's codegen allows limited sync-wait commands per ISA struct
    (e.g. a Matmult's waits all land on the generated LDWEIGHTS struct which
    has one slot). Move excess waits onto same-engine NoOps inserted
    immediately before the instruction - program order on the engine queue
    preserves the synchronization semantics."""
    nfix = 0
    for fn in nc.m.functions:
        for bb in fn.blocks:
            out = []
            for ins in bb.instructions:
                si = ins.sync_info
                if (type(ins).__name__ not in _WAIT_SKIP and si is not None
                        and si.on_wait and len(si.on_wait) > 1):
                    waits = list(si.on_wait)
                    extra, keep = waits[:-1], waits[-1:]
                    for k, w in enumerate(extra):
                        nop = mybir.InstNoOp(name=f"{ins.name}-wf{k}", ins=[],
                                             outs=[])
                        nop.engine = ins.engine
                        nop.sync_info = mybir.SyncInfo(on_wait=[w],
                                                       on_update=[])
                        out.append(nop)
                    ins.sync_info = mybir.SyncInfo(
                        on_wait=keep, on_update=list(si.on_update or []))
                    nfix += 1
                out.append(ins)
            bb.instructions = out
    return nfix


_cached_module = None


def _get_module():
    global _cached_module
    if _cached_module is None:
        nc = build_module()
        # populate .instr bytes for InstCustomDveAnt (reciprocal_approx_fast)
        # - Bacc.compile() runs this pass but the raw-Bass path doesn't, and
        # walrus codegen fails with "ISA wrong length" on empty .instr
        mybir.codegen_inst_isa_subclasses(nc)
        _legalize_waits(nc)
        _cached_module = nc
    return _cached_module


def make_in_maps(x, W_attn, b_attn, W_proj, b_proj):
    import ml_dtypes
    bf = ml_dtypes.bfloat16
    x = np.asarray(x, dtype=np.float32)
    wa = np.asarray(W_attn, dtype=np.float32)
    wq = np.ascontiguousarray(wa[:, 0:C].astype(bf))
    wk = np.ascontiguousarray(wa[:, C:2 * C].astype(bf))
    wv = np.ascontiguousarray(wa[:, 2 * C:3 * C].astype(bf))
    wp = np.ascontiguousarray(np.asarray(W_proj, dtype=np.float32).astype(bf))
    ba = np.ascontiguousarray(
        np.asarray(b_attn, dtype=np.float32).reshape(1, C3))
    bp = np.ascontiguousarray(
        np.asarray(b_proj, dtype=np.float32).reshape(1, C))
    return [
        dict(xT=np.ascontiguousarray(x[b].T.astype(bf)),
             Wq=wq, Wk=wk, Wv=wv, Wp=wp, b_attn=ba, b_proj=bp)
        for b in range(x.shape[0])
    ]


def run(x, W_attn, b_attn, W_proj, b_proj, trace=False, **spmd_kwargs):
    nc = _get_module()
    in_maps = make_in_maps(x, W_attn, b_attn, W_proj, b_proj)
    res = run_bass_kernel_spmd(nc, in_maps, list(range(NCORES)), trace=trace,
                               **spmd_kwargs)
    out = np.stack([res.results[b]["out"] for b in range(len(in_maps))],
                   axis=0)
    return out, res


def kernel(x, W_attn, b_attn, W_proj, b_proj):
    out, _ = run(x, W_attn, b_attn, W_proj, b_proj)
    return out


# revision 17
# speedup vs baseline: 1.5550x; 1.3307x over previous
"""Causal self-attention Trainium2 Bass kernel (V5).

Full-input contract: kernel(**inputs) takes the unsharded inputs
(x [8,1024,768], W_attn [768,2304], b_attn [2304], W_proj [768,768],
b_proj [768]) and returns the full output [8,1024,768].

Sharding: data parallel - batch element b runs on NeuronCore b (B=8 =
n_cores), no collectives needed.

V5 changes vs V4 (trace-driven; V4 331us, PE 60% cov, HAM throttled to
K=4/8 for the entire 188us attention phase, 40us DVE reciprocal, 88us
ACT exp):
  - host-side prep: x is transposed and cast to bf16 on the host
    (xT input [768,1024]); W_attn split into Wq/Wk/Wv and cast bf16;
    W_proj bf16. Kills the 48 PE transposes + DVE copies of phase 1 and
    halves weight DMA bytes.
  - all GEMMs run on bf16 operands (fp32 PSUM accumulation).
  - attention pipeline unit is a (k-tile, col-half) CHUNK with fp32
    scores in a [128, 2x512] PSUM tile (2 banks, both heads of the
    pair). Chunks are double-buffered (4 banks) next to the 4 avp
    banks, so the score MM for chunk n+2 no longer waits on exp(n):
    the PE never idles long enough for HAM to re-throttle.
  - one exp per chunk covers both heads ([128, 2, n] AP) - halves ACT
    instruction count; one affine_select masks both heads' diagonal.
  - softmax divide: l-rows gathered by SBUF->SBUF DMA into a [4,512]
    tile per head pair, ONE reciprocal_approx_fast (the V4 kernel spent
    40us in 12 full-precision Newton reciprocals), DMA broadcast,
    DVE multiply fused into yT (bf16).
"""

import os
import sys

import numpy as np

for _p in ("/opt/trn_rl_repo", "/root/.axon_site/_ro/trn_rl_repo"):
    if os.path.isdir(_p) and _p not in sys.path:
        sys.path.insert(0, _p)
        break

import concourse.bass as bass
import concourse.mybir as mybir
import concourse.tile as tile
from concourse.bass_utils import run_bass_kernel_spmd

T, C, H = 1024, 768, 12
C3 = 3 * C
NCORES = 8
NT = T // 128    # 8 t-tiles
NC_ = C // 128   # 6 c-tiles
NHP = H // 2     # 6 head pairs
f32 = mybir.dt.float32
bf16 = mybir.dt.bfloat16

EXP = mybir.ActivationFunctionType.Exp


def build_module():
    nc = bass.Bass()
    xT_d = nc.dram_tensor("xT", [C, T], bf16, kind="ExternalInput")
    wq_d = nc.dram_tensor("Wq", [C, C], bf16, kind="ExternalInput")
    wk_d = nc.dram_tensor("Wk", [C, C], bf16, kind="ExternalInput")
    wv_d = nc.dram_tensor("Wv", [C, C], bf16, kind="ExternalInput")
    wp_d = nc.dram_tensor("Wp", [C, C], bf16, kind="ExternalInput")
    ba_d = nc.dram_tensor("b_attn", [1, C3], f32, kind="ExternalInput")
    bp_d = nc.dram_tensor("b_proj", [1, C], f32, kind="ExternalInput")
    out_d = nc.dram_tensor("out", [T, C], f32, kind="ExternalOutput")

    with tile.TileContext(nc) as tc:
        with tc.tile_pool(name="persist", bufs=1) as P0:
            qkT = [P0.tile([128, T], bf16, name=f"qkT{m}") for m in range(2 * NC_)]
            # per head: 64 v-dim columns + 64 ones-columns. The AV matmul
            # then emits y rows at partitions 0:64 AND the softmax
            # denominator l replicated across partitions 64:128 - a free
            # partition-broadcast on the PE (MM cost depends only on the
            # moving-operand columns).
            vA = [P0.tile([128, 128 * H], bf16, name=f"vA{t}") for t in range(NT)]
            yT = [P0.tile([128, T], bf16, name=f"yT{c}") for c in range(NC_)]
            ba_sb = P0.tile([1, C], f32, name="ba_sb")
            bp_sb = P0.tile([1, C], f32, name="bp_sb")
            baB = P0.tile([128, C], f32, name="baB")   # b_attn v-part bcast
            bpB = P0.tile([128, C], f32, name="bpB")   # b_proj bcast
            wpt = [P0.tile([128, C], bf16, name=f"wp{c}") for c in range(NC_)]
            bqk = [P0.tile([128, 1], f32, name=f"bqk{m}") for m in range(2 * NC_)]
            warm_src = P0.tile([1, 16], f32, name="warm_src")
            nc.vector.memset(warm_src[:], 1.0)

            # preload the exp table while ACT is idle (else the first
            # attention exp pays the ~2.7us ACT_TABLE_LOAD inline)
            warm = P0.tile([1, 16], f32, name="warm")
            nc.scalar.activation(warm[:], warm_src[:], EXP, scale=0.125)

            # ---- phase A: qkv GEMMs (x arrives pre-transposed bf16) ----
            with tc.tile_pool(name="sbA", bufs=1) as SBA:
                xT = [SBA.tile([128, T], bf16, name=f"xT{c}", tag=f"xT{c}",
                               bufs=1) for c in range(NC_)]
                wV = [SBA.tile([128, C], bf16, name=f"wV{c}", tag=f"wV{c}",
                               bufs=1) for c in range(NC_)]
                # interleave x/weight loads across both HWDGE queues so the
                # first v-GEMM accumulation chain can start ~2 tiles in
                nc.sync.dma_start(out=ba_sb[:], in_=ba_d[0:1, 2 * C:3 * C])
                for c in range(NC_):
                    q = nc.sync if c % 2 == 0 else nc.scalar
                    q.dma_start(out=xT[c][:],
                                in_=xT_d[128 * c:128 * (c + 1), :])
                    q.dma_start(out=wV[c][:],
                                in_=wv_d[128 * c:128 * (c + 1), :])
                # one-time bias broadcast (free-dim stride-0 DMA replicate)
                # on the gpsimd SWDGE queue (descriptor gen on Q7, off both
                # HWDGE queues)
                nc.gpsimd.dma_start(
                    out=baB[:],
                    in_=ba_sb[0:1, :].unsqueeze(1).to_broadcast([1, 128, C]))
                baB_r = baB.rearrange("p (h e) -> p h e", h=H)
                # bqk partition-scatter DMAs (4B-granular, slow to issue) on
                # the gpsimd SWDGE queue, off the weight-load path
                for m in range(2 * NC_):
                    nc.gpsimd.dma_start(
                        out=bqk[m][:],
                        in_=ba_d[0:1, 128 * m:128 * (m + 1)]
                            .rearrange("a p -> p a"))
                # q/k weight loads stream behind the v weights
                wQ = [SBA.tile([128, C], bf16, name=f"wQ{c}", tag=f"wQ{c}",
                               bufs=1) for c in range(NC_)]
                wK = [SBA.tile([128, C], bf16, name=f"wK{c}", tag=f"wK{c}",
                               bufs=1) for c in range(NC_)]
                for c in range(NC_):
                    q = nc.sync if c % 2 == 0 else nc.scalar
                    q.dma_start(out=wQ[c][:],
                                in_=wq_d[128 * c:128 * (c + 1), :])
                for c in range(NC_):
                    q = nc.sync if c % 2 == 0 else nc.scalar
                    q.dma_start(out=wK[c][:],
                                in_=wk_d[128 * c:128 * (c + 1), :])

                with tc.tile_pool(name="psA", bufs=1, space="PSUM") as PSA:
                    # v: stationary xT columns, moving W_v rows
                    for t in range(NT):
                        accv = PSA.tile([128, C], f32, tag="v", bufs=2,
                                        name="accv")
                        for c in range(NC_):
                            xcol = xT[c][:, 128 * t:128 * (t + 1)]
                            nc.tensor.matmul(accv[:, 0:512], xcol,
                                             wV[c][:, 0:512],
                                             start=(c == 0), stop=(c == NC_ - 1))
                            nc.tensor.matmul(accv[:, 512:C], xcol,
                                             wV[c][:, 512:C],
                                             start=(c == 0), stop=(c == NC_ - 1))
                        # per-head layout [ones(64) | v(64)]: the ones FIRST
                        # so the AV matmul puts the replicated l at
                        # partitions 0:64 - the custom-DVE reciprocal ignores
                        # a shifted input partition base, standard TT doesn't
                        av = vA[t].rearrange("p (h e) -> p h e", h=H)
                        nc.vector.memset(av[:, :, 0:64], 1.0)
                        # eviction with fused bias add
                        nc.vector.tensor_tensor(
                            av[:, :, 64:128],
                            accv[:].rearrange("p (h e) -> p h e", h=H),
                            baB_r[:, :, 0:64],
                            mybir.AluOpType.add)

                    # q^T / k^T: out rows = W columns (m-tile), moving xT
                    for half in range(2):
                        wh = wQ if half == 0 else wK
                        for mm in range(NC_):
                            m = NC_ * half + mm
                            acc = PSA.tile([128, T], f32, tag="qk", bufs=2,
                                           name="acc")
                            for c in range(NC_):
                                wa = wh[c][:, 128 * mm:128 * (mm + 1)]
                                for j2 in range(2):
                                    nc.tensor.matmul(
                                        acc[:, 512 * j2:512 * (j2 + 1)],
                                        wa,
                                        xT[c][:, 512 * j2:512 * (j2 + 1)],
                                        start=(c == 0), stop=(c == NC_ - 1),
                                    )
                            # psum -> sbuf(bf16) with per-partition bias add
                            nc.vector.tensor_scalar_add(qkT[m][:], acc[:],
                                                        bqk[m][:])

            # ---- phase B: attention ----
            # W_proj / b_proj loads issued here: sync queue is idle now and
            # phase C needs them much later
            nc.sync.dma_start(out=bp_sb[:], in_=bp_d[:])
            for c in range(NC_):
                q = nc.sync if c % 2 == 0 else nc.scalar
                q.dma_start(out=wpt[c][:],
                            in_=wp_d[128 * c:128 * (c + 1), :])
            nc.gpsimd.dma_start(
                out=bpB[:],
                in_=bp_sb[0:1, :].unsqueeze(1).to_broadcast([1, 128, C]))

            with tc.tile_pool(name="ps3", bufs=1, space="PSUM") as PS3, \
                 tc.tile_pool(name="sb3", bufs=1) as SB3:
                from collections import deque
                pending = deque()   # deferred normalization pipeline stages

                def pop_pending(k=2):
                    n = 0
                    while pending and n < k:
                        s = pending.popleft()
                        if s is not None:
                            s()
                        n += 1

                # chunk list: (i, w); w=0 -> query cols [lo,512) (i<4 only),
                # w=1 -> [max(lo,512), 1024)
                chunks = []
                for i in range(NT):
                    if i < 4:
                        chunks.append((i, 0))
                        chunks.append((i, 1))
                    else:
                        chunks.append((i, 1))

                def chunk_cols(i, w):
                    lo = 128 * i
                    if w == 0:
                        return lo, 512
                    return max(lo, 512), T

                for hp in range(NHP):
                    qt = qkT[hp]
                    kt = qkT[NC_ + hp]
                    sps = {}   # chunk -> score PSUM tile
                    pbs = {}   # chunk -> exp'd SBUF tile
                    avs = {}   # (hs, half) -> [128,512] accumulator

                    def emit_score(ch):
                        i, w = ch
                        lo = 128 * i
                        c0, c1 = chunk_cols(i, w)
                        scp = PS3.tile([128, 1024], f32, tag="sc", bufs=2,
                                       name="scp")
                        for hs in range(2):
                            base = 64 * hs
                            nc.tensor.matmul(
                                scp[:, 512 * hs:512 * hs + (c1 - c0)],
                                kt[base:base + 64, lo:lo + 128],
                                qt[base:base + 64, c0:c1],
                                start=True, stop=True)
                        sps[ch] = scp

                    def emit_exp(ch):
                        i, w = ch
                        lo = 128 * i
                        c0, c1 = chunk_cols(i, w)
                        n = c1 - c0
                        scp = sps.pop(ch)
                        pb = SB3.tile([128, 1024], bf16, tag="pb", bufs=4,
                                      name="pb")
                        scv = scp.rearrange("p (s n) -> p s n", s=2)
                        pbv = pb.rearrange("p (s n) -> p s n", s=2)
                        nc.scalar.activation(pbv[:, :, 0:n], scv[:, :, 0:n],
                                             EXP, scale=0.125)
                        if c0 == lo:
                            # diagonal [128,128] block (both heads):
                            # keep iff q - key >= 0
                            nc.gpsimd.affine_select(
                                out=pbv[:, :, 0:128], in_=pbv[:, :, 0:128],
                                pattern=[[0, 2], [1, 128]],
                                compare_op=mybir.AluOpType.is_ge, fill=0.0,
                                base=0, channel_multiplier=-1,
                            )
                        pbs[ch] = pb

                    def emit_av(ch):
                        i, w = ch
                        c0, c1 = chunk_cols(i, w)
                        n = c1 - c0
                        pb = pbs.pop(ch)
                        if i == 0 and w == 0:
                            for hs in range(2):
                                for half in range(2):
                                    avs[(hs, half)] = PS3.tile(
                                        [128, 512], f32, tag=f"av{hs}{half}",
                                        bufs=1, name=f"av{hs}{half}")
                        for hs in range(2):
                            h = 2 * hp + hs
                            vt = vA[i][:, 128 * h:128 * h + 128]
                            if w == 0:
                                nc.tensor.matmul(
                                    avs[(hs, 0)][:, c0:512], vt,
                                    pb[:, 512 * hs:512 * hs + n],
                                    start=(i == 0), stop=(i == 3),
                                    skip_group_check=True)
                            else:
                                nc.tensor.matmul(
                                    avs[(hs, 1)][:, c0 - 512:512], vt,
                                    pb[:, 512 * hs:512 * hs + n],
                                    start=(i == 0), stop=(i == NT - 1),
                                    skip_group_check=True)

                    def make_norm(myhp, myavs, half):
                        # avs rows 0:64 hold l replicated across partitions
                        # (ones-columns in vA), rows 64:128 hold y.
                        # reciprocal + normalize read PSUM directly - no
                        # staging copies, no DMA gathers/broadcasts.
                        loc = {}

                        def s_recip():
                            for hs in range(2):
                                rli = SB3.tile([64, 512], f32,
                                               tag=f"rli{hs}{half}", bufs=2,
                                               name=f"rli{hs}{half}")
                                nc.vector.reciprocal_approx_fast(
                                    rli[:], myavs[(hs, half)][0:64, 0:512])
                                loc[hs] = rli

                        def s_mult():
                            for hs in range(2):
                                base = 64 * hs
                                nc.vector.tensor_tensor(
                                    yT[myhp][base:base + 64,
                                             512 * half:512 * (half + 1)],
                                    myavs[(hs, half)][64:128, 0:512],
                                    loc[hs][:],
                                    mybir.AluOpType.mult)

                        return [s_recip, s_mult]

                    # software-pipelined emission: scores TWO chunks ahead of
                    # the AV stream so the in-order PE queue always has a
                    # score MM to run while AV(n) waits on exp/affine(n);
                    # deferred norm stages drain in the gaps
                    nch = len(chunks)
                    pop_pending()
                    emit_score(chunks[0])
                    emit_exp(chunks[0])
                    emit_score(chunks[1])
                    emit_exp(chunks[1])
                    for n_ in range(2, nch):
                        pop_pending()
                        emit_score(chunks[n_])
                        emit_exp(chunks[n_])
                        emit_av(chunks[n_ - 2])
                        if chunks[n_ - 2] == (3, 0):
                            pending.extend(make_norm(hp, avs, 0))
                    pop_pending()
                    emit_av(chunks[nch - 2])
                    emit_av(chunks[nch - 1])
                    pending.extend(make_norm(hp, avs, 1))

                # drain the tail (last pair's normalization)
                while pending:
                    s = pending.popleft()
                    if s is not None:
                        s()

            # ---- phase C: out = y^T.T @ W_proj + b_proj ----
            with tc.tile_pool(name="ps4", bufs=2, space="PSUM") as PS4, \
                 tc.tile_pool(name="sb4", bufs=3) as SB4:
                for t in range(NT):
                    acc = PS4.tile([128, C], f32, tag="pj", name="acc")
                    for c in range(NC_):
                        ycol = yT[c][:, 128 * t:128 * (t + 1)]
                        nc.tensor.matmul(acc[:, 0:512], ycol,
                                         wpt[c][:, 0:512],
                                         start=(c == 0), stop=(c == NC_ - 1))
                        nc.tensor.matmul(acc[:, 512:C], ycol,
                                         wpt[c][:, 512:C],
                                         start=(c == 0), stop=(c == NC_ - 1))
                    ot = SB4.tile([128, C], f32, tag="ot", bufs=3, name="ot")
                    nc.vector.tensor_tensor(ot[:], acc[:], bpB[:],
                                            mybir.AluOpType.add)
                    nc.sync.dma_start(out=out_d[128 * t:128 * (t + 1), :],
                                      in_=ot[:])

    return nc


_WAIT_SKIP = {"InstNoOp", "InstEventSemOp", "InstSemaphoreOp",
              "InstPartitionBroadcast", "InstPartitionAllReduce"}


def _legalize_waits(nc):
    """walrus's codegen allows limited sync-wait commands per ISA struct
    (e.g. a Matmult's waits all land on the generated LDWEIGHTS struct which
    has one slot). Move excess waits onto same-engine NoOps inserted
    immediately before the instruction - program order on the engine queue
    preserves the synchronization semantics."""
    nfix = 0
    for fn in nc.m.functions:
        for bb in fn.blocks:
            out = []
            for ins in bb.instructions:
                si = ins.sync_info
                if (type(ins).__name__ not in _WAIT_SKIP and si is not None
                        and si.on_wait and len(si.on_wait) > 1):
                    waits = list(si.on_wait)
                    extra, keep = waits[:-1], waits[-1:]
                    for k, w in enumerate(extra):
                        nop = mybir.InstNoOp(name=f"{ins.name}-wf{k}", ins=[],
                                             outs=[])
                        nop.engine = ins.engine
                        nop.sync_info = mybir.SyncInfo(on_wait=[w],
                                                       on_update=[])
                        out.append(nop)
                    ins.sync_info = mybir.SyncInfo(
                        on_wait=keep, on_update=list(si.on_update or []))
                    nfix += 1
                out.append(ins)
            bb.instructions = out
    return nfix


_cached_module = None


def _get_module():
    global _cached_module
    if _cached_module is None:
        nc = build_module()
        # populate .instr bytes for InstCustomDveAnt (reciprocal_approx_fast)
        # - Bacc.compile() runs this pass but the raw-Bass path doesn't, and
        # walrus codegen fails with "ISA wrong length" on empty .instr
        mybir.codegen_inst_isa_subclasses(nc)
        _legalize_waits(nc)
        _cached_module = nc
    return _cached_module


def make_in_maps(x, W_attn, b_attn, W_proj, b_proj):
    import ml_dtypes
    bf = ml_dtypes.bfloat16
    x = np.asarray(x, dtype=np.float32)
    wa = np.asarray(W_attn, dtype=np.float32)
    wq = np.ascontiguousarray(wa[:, 0:C].astype(bf))
    wk = np.ascontiguousarray(wa[:, C:2 * C].astype(bf))
    wv = np.ascontiguousarray(wa[:, 2 * C:3 * C].astype(bf))
    wp = np.ascontiguousarray(np.asarray(W_proj, dtype=np.float32).astype(bf))
    ba = np.ascontiguousarray(
        np.asarray(b_attn, dtype=np.float32).reshape(1, C3))
    bp = np.ascontiguousarray(
        np.asarray(b_proj, dtype=np.float32).reshape(1, C))
    return [
        dict(xT=np.ascontiguousarray(x[b].T.astype(bf)),
             Wq=wq, Wk=wk, Wv=wv, Wp=wp, b_attn=ba, b_proj=bp)
        for b in range(x.shape[0])
    ]


def run(x, W_attn, b_attn, W_proj, b_proj, trace=False, **spmd_kwargs):
    nc = _get_module()
    in_maps = make_in_maps(x, W_attn, b_attn, W_proj, b_proj)
    res = run_bass_kernel_spmd(nc, in_maps, list(range(NCORES)), trace=trace,
                               **spmd_kwargs)
    out = np.stack([res.results[b]["out"] for b in range(len(in_maps))],
                   axis=0)
    return out, res


def kernel(x, W_attn, b_attn, W_proj, b_proj):
    out, _ = run(x, W_attn, b_attn, W_proj, b_proj)
    return out


# revision 23
# speedup vs baseline: 1.8522x; 1.1911x over previous
"""Causal self-attention Trainium2 Bass kernel (V5).

Full-input contract: kernel(**inputs) takes the unsharded inputs
(x [8,1024,768], W_attn [768,2304], b_attn [2304], W_proj [768,768],
b_proj [768]) and returns the full output [8,1024,768].

Sharding: data parallel - batch element b runs on NeuronCore b (B=8 =
n_cores), no collectives needed.

V5 changes vs V4 (trace-driven; V4 331us, PE 60% cov, HAM throttled to
K=4/8 for the entire 188us attention phase, 40us DVE reciprocal, 88us
ACT exp):
  - host-side prep: x is transposed and cast to bf16 on the host
    (xT input [768,1024]); W_attn split into Wq/Wk/Wv and cast bf16;
    W_proj bf16. Kills the 48 PE transposes + DVE copies of phase 1 and
    halves weight DMA bytes.
  - all GEMMs run on bf16 operands (fp32 PSUM accumulation).
  - attention pipeline unit is a (k-tile, col-half) CHUNK with fp32
    scores in a [128, 2x512] PSUM tile (2 banks, both heads of the
    pair). Chunks are double-buffered (4 banks) next to the 4 avp
    banks, so the score MM for chunk n+2 no longer waits on exp(n):
    the PE never idles long enough for HAM to re-throttle.
  - one exp per chunk covers both heads ([128, 2, n] AP) - halves ACT
    instruction count; one affine_select masks both heads' diagonal.
  - softmax divide: l-rows gathered by SBUF->SBUF DMA into a [4,512]
    tile per head pair, ONE reciprocal_approx_fast (the V4 kernel spent
    40us in 12 full-precision Newton reciprocals), DMA broadcast,
    DVE multiply fused into yT (bf16).
"""

import os
import sys

import numpy as np

for _p in ("/opt/trn_rl_repo", "/root/.axon_site/_ro/trn_rl_repo"):
    if os.path.isdir(_p) and _p not in sys.path:
        sys.path.insert(0, _p)
        break

import concourse.bass as bass
import concourse.mybir as mybir
import concourse.tile as tile
from concourse.bass_utils import run_bass_kernel_spmd

T, C, H = 1024, 768, 12
C3 = 3 * C
NCORES = 8
NT = T // 128    # 8 t-tiles
NC_ = C // 128   # 6 c-tiles
NHP = H // 2     # 6 head pairs
f32 = mybir.dt.float32
bf16 = mybir.dt.bfloat16

EXP = mybir.ActivationFunctionType.Exp


def build_module():
    nc = bass.Bass()
    xT_d = nc.dram_tensor("xT", [C, T], bf16, kind="ExternalInput")
    wq_d = nc.dram_tensor("Wq", [C, C], bf16, kind="ExternalInput")
    wk_d = nc.dram_tensor("Wk", [C, C], bf16, kind="ExternalInput")
    wv_d = nc.dram_tensor("Wv", [C, C], bf16, kind="ExternalInput")
    wp_d = nc.dram_tensor("Wp", [C, C], bf16, kind="ExternalInput")
    ba_d = nc.dram_tensor("b_attn", [1, C3], f32, kind="ExternalInput")
    bp_d = nc.dram_tensor("b_proj", [1, C], f32, kind="ExternalInput")
    out_d = nc.dram_tensor("out", [T, C], f32, kind="ExternalOutput")

    with tile.TileContext(nc) as tc:
        with tc.tile_pool(name="persist", bufs=1) as P0:
            qkT = [P0.tile([128, T], bf16, name=f"qkT{m}") for m in range(2 * NC_)]
            # per head: 64 v-dim columns + 64 ones-columns. The AV matmul
            # then emits y rows at partitions 0:64 AND the softmax
            # denominator l replicated across partitions 64:128 - a free
            # partition-broadcast on the PE (MM cost depends only on the
            # moving-operand columns).
            vA = [P0.tile([128, 128 * H], bf16, name=f"vA{t}") for t in range(NT)]
            yT = [P0.tile([128, T], bf16, name=f"yT{c}") for c in range(NC_)]
            ba_sb = P0.tile([1, C], f32, name="ba_sb")
            bp_sb = P0.tile([1, C], f32, name="bp_sb")
            baB = P0.tile([128, C], f32, name="baB")   # b_attn v-part bcast
            bpB = P0.tile([128, C], f32, name="bpB")   # b_proj bcast
            wpt = [P0.tile([128, C], bf16, name=f"wp{c}") for c in range(NC_)]
            bqk = [P0.tile([128, 1], f32, name=f"bqk{m}") for m in range(2 * NC_)]
            # xT / q / k weights persist into the attention phase: the q^T/k^T
            # GEMM for head pair hp+1 is interleaved into hp's attention
            xT = [P0.tile([128, T], bf16, name=f"xT{c}") for c in range(NC_)]
            wQ = [P0.tile([128, C], bf16, name=f"wQ{c}") for c in range(NC_)]
            wK = [P0.tile([128, C], bf16, name=f"wK{c}") for c in range(NC_)]
            warm_src = P0.tile([1, 16], f32, name="warm_src")
            nc.vector.memset(warm_src[:], 1.0)

            # preload the exp table while ACT is idle (else the first
            # attention exp pays the ~2.7us ACT_TABLE_LOAD inline)
            warm = P0.tile([1, 16], f32, name="warm")
            nc.scalar.activation(warm[:], warm_src[:], EXP, scale=0.125)

            # ---- phase A: v GEMM (x arrives pre-transposed bf16) ----
            with tc.tile_pool(name="sbA", bufs=1) as SBA:
                wV = [SBA.tile([128, C], bf16, name=f"wV{c}", tag=f"wV{c}",
                               bufs=1) for c in range(NC_)]
                # interleave x/weight loads across both HWDGE queues so the
                # first v-GEMM accumulation chain can start ~2 tiles in
                nc.sync.dma_start(out=ba_sb[:], in_=ba_d[0:1, 2 * C:3 * C])
                for c in range(NC_):
                    q = nc.sync if c % 2 == 0 else nc.scalar
                    q.dma_start(out=xT[c][:],
                                in_=xT_d[128 * c:128 * (c + 1), :])
                    q.dma_start(out=wV[c][:],
                                in_=wv_d[128 * c:128 * (c + 1), :])
                # one-time bias broadcast (free-dim stride-0 DMA replicate)
                # on the gpsimd SWDGE queue (descriptor gen on Q7, off both
                # HWDGE queues)
                nc.gpsimd.dma_start(
                    out=baB[:],
                    in_=ba_sb[0:1, :].unsqueeze(1).to_broadcast([1, 128, C]))
                baB_r = baB.rearrange("p (h e) -> p h e", h=H)
                # bqk partition-scatter DMAs (4B-granular, slow to issue) on
                # the gpsimd SWDGE queue, off the weight-load path
                for m in range(2 * NC_):
                    nc.gpsimd.dma_start(
                        out=bqk[m][:],
                        in_=ba_d[0:1, 128 * m:128 * (m + 1)]
                            .rearrange("a p -> p a"))
                # q/k weight loads stream behind the v weights
                for c in range(NC_):
                    q = nc.sync if c % 2 == 0 else nc.scalar
                    q.dma_start(out=wQ[c][:],
                                in_=wq_d[128 * c:128 * (c + 1), :])
                for c in range(NC_):
                    q = nc.sync if c % 2 == 0 else nc.scalar
                    q.dma_start(out=wK[c][:],
                                in_=wk_d[128 * c:128 * (c + 1), :])

                with tc.tile_pool(name="psA", bufs=1, space="PSUM") as PSA:
                    # v: stationary xT columns, moving W_v rows
                    for t in range(NT):
                        accv = PSA.tile([128, C], f32, tag="v", bufs=2,
                                        name="accv")
                        for c in range(NC_):
                            xcol = xT[c][:, 128 * t:128 * (t + 1)]
                            nc.tensor.matmul(accv[:, 0:512], xcol,
                                             wV[c][:, 0:512],
                                             start=(c == 0), stop=(c == NC_ - 1))
                            nc.tensor.matmul(accv[:, 512:C], xcol,
                                             wV[c][:, 512:C],
                                             start=(c == 0), stop=(c == NC_ - 1))
                        # per-head layout [ones(64) | v(64)]: the ones FIRST
                        # so the AV matmul puts the replicated l at
                        # partitions 0:64 - the custom-DVE reciprocal ignores
                        # a shifted input partition base, standard TT doesn't
                        av = vA[t].rearrange("p (h e) -> p h e", h=H)
                        nc.vector.memset(av[:, :, 0:64], 1.0)
                        # eviction with fused bias add
                        nc.vector.tensor_tensor(
                            av[:, :, 64:128],
                            accv[:].rearrange("p (h e) -> p h e", h=H),
                            baB_r[:, :, 0:64],
                            mybir.AluOpType.add)

            # ---- phase B: attention with interleaved q^T/k^T GEMMs ----
            # W_proj / b_proj loads issued here: sync queue is idle now and
            # phase C needs them much later
            nc.sync.dma_start(out=bp_sb[:], in_=bp_d[:])
            for c in range(NC_):
                q = nc.sync if c % 2 == 0 else nc.scalar
                q.dma_start(out=wpt[c][:],
                            in_=wp_d[128 * c:128 * (c + 1), :])
            nc.gpsimd.dma_start(
                out=bpB[:],
                in_=bp_sb[0:1, :].unsqueeze(1).to_broadcast([1, 128, C]))

            with tc.tile_pool(name="ps3", bufs=1, space="PSUM") as PS3, \
                 tc.tile_pool(name="sb3", bufs=1) as SB3:
                from collections import deque
                pending = deque()   # deferred normalization pipeline stages

                def pop_pending(k=2):
                    n = 0
                    while pending and n < k:
                        s = pending.popleft()
                        if s is not None:
                            s()
                        n += 1

                # chunk list: (i, w); w=0 -> query cols [lo,512) (i<4 only),
                # w=1 -> [max(lo,512), 1024)
                chunks = []
                for i in range(NT):
                    if i < 4:
                        chunks.append((i, 0))
                        chunks.append((i, 1))
                    else:
                        chunks.append((i, 1))

                def chunk_cols(i, w):
                    lo = 128 * i
                    if w == 0:
                        return lo, 512
                    return max(lo, 512), T

                def emit_qk_job(tp, jidx, tags):
                    # one [128,512] slice of q^T (jidx 0/1) or k^T (2/3) for
                    # target head pair tp; the accumulator borrows an
                    # avs-tagged PSUM bank (free between L-normalization and
                    # the next pair's AV allocation)
                    m = tp if jidx < 2 else NC_ + tp
                    j2 = jidx % 2
                    wh = wQ if m < NC_ else wK
                    mm = m % NC_
                    acc = PS3.tile([128, 512], f32, tag=tags[jidx], bufs=1,
                                   name="qka")
                    for c in range(NC_):
                        nc.tensor.matmul(
                            acc[:], wh[c][:, 128 * mm:128 * (mm + 1)],
                            xT[c][:, 512 * j2:512 * (j2 + 1)],
                            start=(c == 0), stop=(c == NC_ - 1))
                    # psum -> sbuf(bf16) with per-partition bias add
                    nc.vector.tensor_scalar_add(
                        qkT[m][:, 512 * j2:512 * (j2 + 1)], acc[:],
                        bqk[m][:])

                # prelude: q/k for head pair 0 (all four avs-tag banks are
                # free before the first AV allocation)
                for jidx in range(4):
                    emit_qk_job(0, jidx, ["av00", "av10", "av01", "av11"])
                JTAGS = ["av00", "av10", "av00", "av10"]

                for hp in range(NHP):
                    qt = qkT[hp]
                    kt = qkT[NC_ + hp]
                    sps = {}   # chunk -> score PSUM tile
                    pbs = {}   # chunk -> exp'd SBUF tile
                    avs = {}   # (hs, half) -> [128,512] accumulator

                    def emit_score(ch):
                        i, w = ch
                        lo = 128 * i
                        c0, c1 = chunk_cols(i, w)
                        scp = PS3.tile([128, 1024], f32, tag="sc", bufs=2,
                                       name="scp")
                        for hs in range(2):
                            base = 64 * hs
                            nc.tensor.matmul(
                                scp[:, 512 * hs:512 * hs + (c1 - c0)],
                                kt[base:base + 64, lo:lo + 128],
                                qt[base:base + 64, c0:c1],
                                start=True, stop=True)
                        sps[ch] = scp

                    def emit_exp(ch):
                        i, w = ch
                        lo = 128 * i
                        c0, c1 = chunk_cols(i, w)
                        n = c1 - c0
                        scp = sps.pop(ch)
                        pb = SB3.tile([128, 1024], bf16, tag="pb", bufs=4,
                                      name="pb")
                        scv = scp.rearrange("p (s n) -> p s n", s=2)
                        pbv = pb.rearrange("p (s n) -> p s n", s=2)
                        nc.scalar.activation(pbv[:, :, 0:n], scv[:, :, 0:n],
                                             EXP, scale=0.125)
                        if c0 == lo:
                            # diagonal [128,128] block (both heads):
                            # keep iff q - key >= 0
                            nc.gpsimd.affine_select(
                                out=pbv[:, :, 0:128], in_=pbv[:, :, 0:128],
                                pattern=[[0, 2], [1, 128]],
                                compare_op=mybir.AluOpType.is_ge, fill=0.0,
                                base=0, channel_multiplier=-1,
                            )
                        pbs[ch] = pb

                    def emit_av(ch):
                        i, w = ch
                        c0, c1 = chunk_cols(i, w)
                        n = c1 - c0
                        pb = pbs.pop(ch)
                        if i == 0 and w == 0:
                            for hs in range(2):
                                for half in range(2):
                                    avs[(hs, half)] = PS3.tile(
                                        [128, 512], f32, tag=f"av{hs}{half}",
                                        bufs=1, name=f"av{hs}{half}")
                        for hs in range(2):
                            h = 2 * hp + hs
                            vt = vA[i][:, 128 * h:128 * h + 128]
                            if w == 0:
                                nc.tensor.matmul(
                                    avs[(hs, 0)][:, c0:512], vt,
                                    pb[:, 512 * hs:512 * hs + n],
                                    start=(i == 0), stop=(i == 3),
                                    skip_group_check=True)
                            else:
                                nc.tensor.matmul(
                                    avs[(hs, 1)][:, c0 - 512:512], vt,
                                    pb[:, 512 * hs:512 * hs + n],
                                    start=(i == 0), stop=(i == NT - 1),
                                    skip_group_check=True)

                    def make_norm(myhp, myavs, half):
                        # avs rows 0:64 hold l replicated across partitions
                        # (ones-columns in vA), rows 64:128 hold y.
                        # reciprocal + normalize read PSUM directly - no
                        # staging copies, no DMA gathers/broadcasts.
                        loc = {}

                        def s_recip():
                            for hs in range(2):
                                rli = SB3.tile([64, 512], f32,
                                               tag=f"rli{hs}{half}", bufs=2,
                                               name=f"rli{hs}{half}")
                                nc.vector.reciprocal_approx_fast(
                                    rli[:], myavs[(hs, half)][0:64, 0:512])
                                loc[hs] = rli

                        def s_mult():
                            for hs in range(2):
                                base = 64 * hs
                                nc.vector.tensor_tensor(
                                    yT[myhp][base:base + 64,
                                             512 * half:512 * (half + 1)],
                                    myavs[(hs, half)][64:128, 0:512],
                                    loc[hs][:],
                                    mybir.AluOpType.mult)

                        return [s_recip, s_mult]

                    # software-pipelined emission: scores TWO chunks ahead of
                    # the AV stream so the in-order PE queue always has a
                    # score MM to run while AV(n) waits on exp/affine(n);
                    # deferred norm stages drain in the gaps
                    nch = len(chunks)
                    pop_pending()
                    emit_score(chunks[0])
                    emit_exp(chunks[0])
                    emit_score(chunks[1])
                    emit_exp(chunks[1])
                    for n_ in range(2, nch):
                        pop_pending()
                        emit_score(chunks[n_])
                        emit_exp(chunks[n_])
                        emit_av(chunks[n_ - 2])
                        if n_ == 8:
                            # L-half normalization inline: the avs L banks
                            # are re-tagged as qk accumulators right after
                            for s in make_norm(hp, avs, 0):
                                s()
                        elif n_ >= 9 and hp + 1 < NHP:
                            emit_qk_job(hp + 1, n_ - 9, JTAGS)
                    pop_pending()
                    emit_av(chunks[nch - 2])
                    emit_av(chunks[nch - 1])
                    if hp + 1 < NHP:
                        emit_qk_job(hp + 1, 3, JTAGS)
                    pending.extend(make_norm(hp, avs, 1))

                # drain the tail (last pair's normalization)
                while pending:
                    s = pending.popleft()
                    if s is not None:
                        s()

            # ---- phase C: out = y^T.T @ W_proj + b_proj ----
            with tc.tile_pool(name="ps4", bufs=2, space="PSUM") as PS4, \
                 tc.tile_pool(name="sb4", bufs=3) as SB4:
                for t in range(NT):
                    acc = PS4.tile([128, C], f32, tag="pj", name="acc")
                    for c in range(NC_):
                        ycol = yT[c][:, 128 * t:128 * (t + 1)]
                        nc.tensor.matmul(acc[:, 0:512], ycol,
                                         wpt[c][:, 0:512],
                                         start=(c == 0), stop=(c == NC_ - 1))
                        nc.tensor.matmul(acc[:, 512:C], ycol,
                                         wpt[c][:, 512:C],
                                         start=(c == 0), stop=(c == NC_ - 1))
                    ot = SB4.tile([128, C], f32, tag="ot", bufs=3, name="ot")
                    nc.vector.tensor_tensor(ot[:], acc[:], bpB[:],
                                            mybir.AluOpType.add)
                    nc.sync.dma_start(out=out_d[128 * t:128 * (t + 1), :],
                                      in_=ot[:])

    return nc


_WAIT_SKIP = {"InstNoOp", "InstEventSemOp", "InstSemaphoreOp",
              "InstPartitionBroadcast", "InstPartitionAllReduce"}


def _legalize_waits(nc):
    """walrus's codegen allows limited sync-wait commands per ISA struct
    (e.g. a Matmult's waits all land on the generated LDWEIGHTS struct which
    has one slot). Move excess waits onto same-engine NoOps inserted
    immediately before the instruction - program order on the engine queue
    preserves the synchronization semantics."""
    nfix = 0
    for fn in nc.m.functions:
        for bb in fn.blocks:
            out = []
            for ins in bb.instructions:
                si = ins.sync_info
                if (type(ins).__name__ not in _WAIT_SKIP and si is not None
                        and si.on_wait and len(si.on_wait) > 1):
                    waits = list(si.on_wait)
                    extra, keep = waits[:-1], waits[-1:]
                    for k, w in enumerate(extra):
                        nop = mybir.InstNoOp(name=f"{ins.name}-wf{k}", ins=[],
                                             outs=[])
                        nop.engine = ins.engine
                        nop.sync_info = mybir.SyncInfo(on_wait=[w],
                                                       on_update=[])
                        out.append(nop)
                    ins.sync_info = mybir.SyncInfo(
                        on_wait=keep, on_update=list(si.on_update or []))
                    nfix += 1
                out.append(ins)
            bb.instructions = out
    return nfix


_cached_module = None


def _get_module():
    global _cached_module
    if _cached_module is None:
        nc = build_module()
        # populate .instr bytes for InstCustomDveAnt (reciprocal_approx_fast)
        # - Bacc.compile() runs this pass but the raw-Bass path doesn't, and
        # walrus codegen fails with "ISA wrong length" on empty .instr
        mybir.codegen_inst_isa_subclasses(nc)
        _legalize_waits(nc)
        _cached_module = nc
    return _cached_module


def make_in_maps(x, W_attn, b_attn, W_proj, b_proj):
    import ml_dtypes
    bf = ml_dtypes.bfloat16
    x = np.asarray(x, dtype=np.float32)
    wa = np.asarray(W_attn, dtype=np.float32)
    wq = np.ascontiguousarray(wa[:, 0:C].astype(bf))
    wk = np.ascontiguousarray(wa[:, C:2 * C].astype(bf))
    wv = np.ascontiguousarray(wa[:, 2 * C:3 * C].astype(bf))
    wp = np.ascontiguousarray(np.asarray(W_proj, dtype=np.float32).astype(bf))
    ba = np.ascontiguousarray(
        np.asarray(b_attn, dtype=np.float32).reshape(1, C3))
    bp = np.ascontiguousarray(
        np.asarray(b_proj, dtype=np.float32).reshape(1, C))
    return [
        dict(xT=np.ascontiguousarray(x[b].T.astype(bf)),
             Wq=wq, Wk=wk, Wv=wv, Wp=wp, b_attn=ba, b_proj=bp)
        for b in range(x.shape[0])
    ]


def run(x, W_attn, b_attn, W_proj, b_proj, trace=False, **spmd_kwargs):
    nc = _get_module()
    in_maps = make_in_maps(x, W_attn, b_attn, W_proj, b_proj)
    res = run_bass_kernel_spmd(nc, in_maps, list(range(NCORES)), trace=trace,
                               **spmd_kwargs)
    out = np.stack([res.results[b]["out"] for b in range(len(in_maps))],
                   axis=0)
    return out, res


def kernel(x, W_attn, b_attn, W_proj, b_proj):
    out, _ = run(x, W_attn, b_attn, W_proj, b_proj)
    return out
